# revision 12
# baseline (speedup 1.0000x reference)
"""CfC (closed-form continuous-time) cell kernel for Trainium2, 8 NeuronCores.

Reference computation (B=8192, IN=256, H=512, all fp32):
    g     = sigmoid(x @ W_gx.T + b_gx + h @ W_gh.T + gate_b)        [B, H]
    f     = tanh(cat([x, h]) @ W_backbone.T + b_backbone)           [B, H]
    tau   = softplus(log_tau) + |g|          (g in (0,1) so |g| == g)
    decay = exp(-delta_t[:, None] * tau)
    out   = decay * h + (1 - decay) * f

Strategy: data-parallel over B (1024 rows per core), weights replicated.
Device work is feature-major (activations ship as xh^T [768, B_shard]) so the
contraction dim lands on SBUF partitions with no on-device transposes.

Precision split (tolerance is 2e-2; measured in numpy against the reference):
  - gate matmul: fp8 e4m3 DoubleRow (2 MACs/cell/cycle).  Gate error is
    attenuated through d(out)/d(zg) = 0.25*dt*decay*(h-f)*sech^2 -> ~6e-3.
  - backbone matmul: fp16 (error enters the output linearly; fp8 here would
    blow the budget - measured 1.9-2.4e-2).
  - elementwise chain: fp16 (DVE 16-bit runs 2x), output fp16.
  Combined measured rel err ~1.2e-2.

Pipeline: loop over 4 output-feature tiles j of 128; each j owns a gate and a
backbone PSUM accumulator [128,1024] (2 banks each); bufs=2 pools fill all 8
banks so PE double-buffers across j.  Gate matmuls are emitted g-supertile-
outer so the first 256KB of fp8 activations unblocks 8 matmuls.  All inputs
stream on one queue in consumption order: wg8, xh8, (ndt, consts), then
xh16 k-tile interleaved with wb16 k-tile so backbone j0 can start before the
full stream lands.  sigmoid via tanh (Sigmoid/Exp never share an ACT table);
softplus(log_tau)+0.5 precomputed on host; a dummy Exp on a [1,8] tile is the
first scalar-engine instruction so the one ACT table load overlaps the DMA
lead-in instead of stalling the first real activation.
"""

from contextlib import ExitStack

import ml_dtypes
import numpy as np

import concourse.bass as bass
import concourse.mybir as mybir
import concourse.tile as tile
from concourse import bacc
from concourse.bass_utils import run_bass_kernel_spmd

B, IN, H = 8192, 256, 512
NCORES = 8
BS = B // NCORES          # 1024 batch rows per core
KIN = IN + H              # 768 contraction dim
KT = KIN // 128           # 6 k-tiles
NG = KT // 2              # 3 fp8 DoubleRow super-tiles (256 contraction each)
NJ = H // 128             # 4 output feature tiles
NCHUNK = 512              # matmul moving free dim per PSUM bank
NCH = BS // NCHUNK        # 2 b-chunks per core

F32 = mybir.dt.float32
FP16 = mybir.dt.float16
FP8 = mybir.dt.float8e4
AF = mybir.ActivationFunctionType
OP = mybir.AluOpType
DR = mybir.MatmulPerfMode.DoubleRow

TRACE = False             # test.py flips this for profiled runs
LAST_RESULT = None        # BassKernelResults of the most recent run

_NC_CACHE = None


def _body(tc, xh16, xh8, wg8, wb16, consts, negdt, outP):
    nc = tc.nc
    with ExitStack() as ctx:
        singles = ctx.enter_context(tc.tile_pool(name="singles", bufs=1))
        work = ctx.enter_context(tc.tile_pool(name="work", bufs=2))
        # One PSUM pool, 4 rotating slots of [128,1024] f32 (2 banks each =
        # all 8 banks).  Allocation order zg0..zg3, zf0..zf3 makes zf_j reuse
        # zg_j's banks, which the gate activation has freed by then.
        ps = ctx.enter_context(tc.tile_pool(name="ps", bufs=4, space="PSUM"))

        xh16_sb = singles.tile([128, KT, BS], FP16, tag="xh16")
        xh8_sb = singles.tile([128, KT, BS], FP8, tag="xh8")
        wg8_sb = singles.tile([128, KT, H], FP8, tag="wg8")
        wb16_sb = singles.tile([128, KT, H], FP16, tag="wb16")
        cst = singles.tile([128, 3, NJ], F32, tag="cst")
        ndt = singles.tile([128, BS], FP16, tag="ndt")

        # ACT table preload: make a trivial Exp the first scalar-queue
        # instruction so the Tanh/Exp table DMA overlaps the input stream.
        d0 = singles.tile([1, 8], F32, tag="d0")
        d1 = singles.tile([1, 8], F32, tag="d1")
        nc.gpsimd.memset(d0, 0.0)
        nc.scalar.activation(out=d1, in_=d0, func=AF.Exp)

        # Input DMAs split across the two HWDGE queues (issue cost ~650ns per
        # DMA serializes per queue), each in PE consumption order.
        xh8_v = xh8.rearrange("p (g c) -> p g c", g=NG)
        xh16_v = xh16.rearrange("p (g c) -> p g c", g=NG)
        wg8_v = wg8.rearrange("p (g c) -> p g c", g=NG)
        for g in range(NG):
            nc.sync.dma_start(
                out=xh8_sb[:, 2 * g:2 * g + 2, :].rearrange("p k b -> p (k b)"),
                in_=xh8_v[:, g, :],
            )
        for g in range(NG):
            nc.sync.dma_start(
                out=xh16_sb[:, 2 * g:2 * g + 2, :].rearrange("p k b -> p (k b)"),
                in_=xh16_v[:, g, :],
            )
        for g in range(NG):
            nc.scalar.dma_start(
                out=wg8_sb[:, 2 * g:2 * g + 2, :].rearrange("p k n -> p (k n)"),
                in_=wg8_v[:, g, :],
            )
        nc.scalar.dma_start(
            out=wb16_sb.rearrange("p k n -> p (k n)"), in_=wb16
        )
        nc.scalar.dma_start(out=ndt, in_=negdt)
        nc.scalar.dma_start(
            out=cst, in_=consts.rearrange("(c j p) -> p c j", p=128, j=NJ)
        )

        zg = {}
        for j in range(NJ):
            zg[j] = ps.tile([128, BS], F32, tag="ps", name=f"zg{j}")

        # PE warm-up: ~24 tiny matmuls on a zeroed tile keep the PE busy
        # through the DMA lead-in so the HAM clock gate reaches 2.4 GHz
        # before the first real matmul (else the first ~3.4us run at 1.2).
        # They scribble on zg0, whose first real matmul resets the bank.
        warm = singles.tile([128, 64], FP16, tag="warm")
        nc.gpsimd.memset(warm, 0.0)
        for _ in range(24):
            nc.tensor.matmul(
                zg[0][0:64, 0:64], warm, warm, start=True, stop=True,
                skip_group_check=True,
            )

        # Gate matmuls, g-supertile outer: each fp8 super-tile arrival
        # unblocks 8 DoubleRow matmuls across all j.
        for g in range(NG):
            for j in range(NJ):
                for n in range(NCH):
                    bsl = slice(n * NCHUNK, (n + 1) * NCHUNK)
                    nc.tensor.matmul(
                        zg[j][:, bsl],
                        wg8_sb[:, 2 * g:2 * g + 2, j * 128:(j + 1) * 128],
                        xh8_sb[:, 2 * g:2 * g + 2, bsl],
                        start=(g == 0),
                        stop=(g == NG - 1),
                        perf_mode=DR,
                    )

        # Per-j: gate activation chain (frees zg banks for the next pair of
        # j's backbone accumulators), backbone matmuls, combine, store.
        decays = {}
        for j in range(NJ):
            tg = work.tile([128, BS], FP16, tag="tg", name=f"tg{j}")
            tau = work.tile([128, BS], FP16, tag="tau", name=f"tau{j}")
            t = work.tile([128, BS], FP16, tag="t", name=f"t{j}")
            # decay_j is consumed late (by p_j after the backbone matmuls);
            # all four must stay live or the ACT queue deadlocks on reuse.
            decay = work.tile([128, BS], FP16, tag="decay", name=f"decay{j}",
                              bufs=NJ)
            # sigmoid(zg + bg) = 0.5 + 0.5*tanh(0.5*zg + bg/2); cst slot 0
            # holds bg/2.  tau = g + softplus = 0.5*tg + (softplus + 0.5).
            nc.scalar.activation(
                out=tg, in_=zg[j], func=AF.Tanh, bias=cst[:, 0, j:j + 1],
                scale=0.5,
            )
            nc.vector.tensor_scalar(
                out=tau, in0=tg, scalar1=0.5, scalar2=cst[:, 2, j:j + 1],
                op0=OP.mult, op1=OP.add,
            )
            nc.vector.tensor_mul(out=t, in0=tau, in1=ndt)
            nc.scalar.activation(out=decay, in_=t, func=AF.Exp)
            decays[j] = decay

        for j in range(NJ):
            zf = ps.tile([128, BS], F32, tag="ps", name=f"zf{j}")
            for n in range(NCH):
                bsl = slice(n * NCHUNK, (n + 1) * NCHUNK)
                for k in range(KT):
                    nc.tensor.matmul(
                        zf[:, bsl],
                        wb16_sb[:, k, j * 128:(j + 1) * 128],
                        xh16_sb[:, k, bsl],
                        start=(k == 0),
                        stop=(k == KT - 1),
                    )
            # Tail chunked per b-half so the post-matmul drain chain after the
            # last matmul is half-sized; outputs go out on the (by now idle)
            # sync HWDGE queue.
            for n in range(NCH):
                bsl = slice(n * NCHUNK, (n + 1) * NCHUNK)
                f = work.tile([128, NCHUNK], FP16, tag="f", name=f"f{j}_{n}")
                hmf = work.tile([128, NCHUNK], FP16, tag="hmf",
                                name=f"hmf{j}_{n}")
                p = work.tile([128, NCHUNK], FP16, tag="p", name=f"p{j}_{n}")
                o = work.tile([128, NCHUNK], FP16, tag="o", name=f"o{j}_{n}")
                nc.scalar.activation(
                    out=f, in_=zf[:, bsl], func=AF.Tanh, bias=cst[:, 1, j:j + 1]
                )
                # out = f + decay * (h - f); h rows live in xh16 k-tiles 2..5
                nc.vector.tensor_sub(out=hmf, in0=xh16_sb[:, 2 + j, bsl], in1=f)
                nc.vector.tensor_mul(out=p, in0=decays[j][:, bsl], in1=hmf)
                nc.vector.tensor_add(out=o, in0=p, in1=f)
                nc.sync.dma_start(
                    out=outP[:, j * BS + n * NCHUNK:j * BS + (n + 1) * NCHUNK],
                    in_=o,
                )


def build_nc():
    nc = bacc.Bacc(
        "TRN2",
        target_bir_lowering=False,
        debug=False,
        enable_asserts=False,
        num_devices=NCORES,
    )
    # Partition-major packed streams: row p holds that partition's entire
    # contiguous payload.
    xh16 = nc.dram_tensor("xh16", [128, KT * BS], FP16, kind="ExternalInput").ap()
    xh8 = nc.dram_tensor("xh8", [128, KT * BS], FP8, kind="ExternalInput").ap()
    wg8 = nc.dram_tensor("wg8", [128, KT * H], FP8, kind="ExternalInput").ap()
    wb16 = nc.dram_tensor("wb16", [128, KT * H], FP16, kind="ExternalInput").ap()
    consts = nc.dram_tensor("consts", [3 * H], F32, kind="ExternalInput").ap()
    negdt = nc.dram_tensor("negdt", [128, BS], FP16, kind="ExternalInput").ap()
    outP = nc.dram_tensor("outP", [128, NJ * BS], FP16, kind="ExternalOutput").ap()
    with tile.TileContext(nc) as tc:
        _body(tc, xh16, xh8, wg8, wb16, consts, negdt, outP)
    nc.compile()
    return nc


def _get_nc():
    global _NC_CACHE
    if _NC_CACHE is None:
        _NC_CACHE = build_nc()
    return _NC_CACHE


def _pack_pmajor(a, kt):
    """[kt*128, C] -> [128, kt*C]: partition-major pack so each of the 128
    DMA rows is contiguous in DRAM."""
    c = a.shape[1]
    return np.ascontiguousarray(
        a.reshape(kt, 128, c).transpose(1, 0, 2).reshape(128, kt * c)
    )


def make_in_maps(x, h, delta_t, W_backbone, b_backbone, W_gx, b_gx, W_gh,
                 gate_b, log_tau):
    f32 = np.float32
    xh = np.concatenate(
        [np.asarray(x, f32), np.asarray(h, f32)], axis=1
    )                                                   # [B, 768]
    xhT = np.ascontiguousarray(xh.T)                    # [768, B] f32
    xhT16 = xhT.astype(np.float16)
    xhT8 = xhT.astype(ml_dtypes.float8_e4m3)
    WgT = np.concatenate(
        [np.asarray(W_gx, f32), np.asarray(W_gh, f32)], axis=1
    ).T                                                 # [768, H]
    wg8_p = _pack_pmajor(WgT.astype(ml_dtypes.float8_e4m3), KT)
    wb16_p = _pack_pmajor(
        np.asarray(W_backbone, f32).T.astype(np.float16), KT
    )
    lt = np.asarray(log_tau, np.float64)
    stau = (np.log1p(np.exp(lt)) + 0.5).astype(f32)
    consts = np.concatenate(
        [
            (np.asarray(b_gx, f32) + np.asarray(gate_b, f32)) * 0.5,
            np.asarray(b_backbone, f32),
            stau,
        ]
    ).astype(f32)                                       # [3H]
    negdt = (-np.asarray(delta_t, f32)).astype(np.float16)   # [B]

    in_maps = []
    for c in range(NCORES):
        sl = slice(c * BS, (c + 1) * BS)
        in_maps.append(
            {
                "xh16": _pack_pmajor(xhT16[:, sl], KT),
                "xh8": _pack_pmajor(xhT8[:, sl], KT),
                "wg8": wg8_p,
                "wb16": wb16_p,
                "consts": consts,
                "negdt": np.ascontiguousarray(
                    np.broadcast_to(negdt[sl][None, :], (128, BS))
                ),
            }
        )
    return in_maps


def kernel(x, h, delta_t, W_backbone, b_backbone, W_gx, b_gx, W_gh, gate_b,
           log_tau):
    global LAST_RESULT
    in_maps = make_in_maps(x, h, delta_t, W_backbone, b_backbone, W_gx, b_gx,
                           W_gh, gate_b, log_tau)
    nc = _get_nc()
    res = run_bass_kernel_spmd(
        nc, in_maps, core_ids=list(range(NCORES)), trace=TRACE
    )
    LAST_RESULT = res
    # outP is [128, NJ*BS] fp16 partition-major; unpack to [H, BS], gather.
    outs = []
    for r in res.results:
        op = np.asarray(r["outP"], dtype=np.float32)
        op = op.reshape(128, NJ, BS).transpose(1, 0, 2).reshape(H, BS)
        outs.append(op)
    out = np.concatenate(outs, axis=1).T
    return np.ascontiguousarray(out).astype(np.float32)
